# revision 15
# baseline (speedup 1.0000x reference)
"""GroupedExperts (MoE bmm path) forward on 8 Trainium2 NeuronCores.

Reference (per expert e):
    h   = silu(x[e] @ w1[e]) * (x[e] @ w3[e])
    out = h @ w2[e]
with E=8, T=4096, D=2048, H=1024, fp32 inputs.

Sharding: expert-parallel - core e owns expert e (no cross-core traffic).

Device kernel design (per core):
  Host stages inputs as bf16 with x pre-transposed to xT [D, T] so every
  matmul consumes its natural layout (no on-device transposes):
    m1/m2: aT/bT[hm, tblk] = sum_dk w1/w3[dk, hm].T @ xT[dk, tblk]
           (lhsT = weight tile [128(D) x 128(H)], rhs = xT tile)
    hT    = silu(aT) * bT                     (ACT + DVE, bf16 result)
    m3:    out[tm, dn] = sum_hk hT[hk, tm].T @ w2[hk, dn]
  PSUM accumulates in fp32; out is written fp32 in natural [T, D] layout.
  Weights stay SBUF-resident (~96KB/partition); xT / hT are double-buffered
  per 512-token block so DMA and PE overlap.

  LDWEIGHTS amortization: the PE pays ~50-100 ns per LDWEIGHTS serially
  (not hidden behind matmuls on TRN2), so the loops are structured to
  reuse each stationary operand for two matmuls:
   - m1/m2 process token blocks in pairs: the two blocks' accumulation
     chains interleave per contraction tile (same weight, two rhs
     tiles, two PSUM banks).
   - m3 streams all four 512-wide D chunks per hT stationary slice
     (4 matmuls per LDWEIGHTS), alternating between the two PSUM bank
     quads so DVE evacuation overlaps the next group's chains.
  After Tile compilation, `_dedupe_ldweights` drops the redundant
  (sync-free, identical-AP) InstLdweights - the PE keeps the
  stationary operand resident across consecutive matmuls.
  PSUM budget: all 8 banks, time-multiplexed between the m1/m2 chains
  (b0-b3) and the m3 output groups (alternating b0-b3 / b4-b7).
  SBUF ~204KB/partition (weights 96 + x 64 + h 32 + staging 12).
"""

import numpy as np
import ml_dtypes

import concourse.bass as bass
import concourse.mybir as mybir
import concourse.tile as tile
from concourse import bacc
from concourse.bass_utils import run_bass_kernel_spmd

E, T, D, H = 8, 4096, 2048, 1024
NCORES = 8
P = 128               # partition dim
TBLK = 512            # token block = moving free dim for m1/m2
NTBLK = T // TBLK     # 8
NDK = D // P          # 16 contraction tiles over D
NHM = H // P          # 8 tiles over H
DBLK = 512            # D chunk = moving free dim for m3
NDN = D // DBLK       # 4
NTSUB = TBLK // P     # 4

BF16 = mybir.dt.bfloat16
F32 = mybir.dt.float32

_CACHE: dict = {}


def _dedupe_ldweights(nc):
    """Drop back-to-back InstLdweights with identical weight APs.

    The PE array keeps the stationary operand resident across matmuls;
    a second LDWEIGHTS that (a) loads the same AP as the most recent
    one, (b) carries no semaphore waits/updates, and (c) has no
    intervening weight-clobbering instruction is a no-op and only costs
    PE issue time. Safe only at the final (post-schedule) order.
    """
    removed = 0
    for blk in nc.m.functions[0].blocks:
        insts = blk.instructions
        last_sig = None
        drop = []
        for idx, inst in enumerate(insts):
            nm = inst.__class__.__name__
            if nm == "InstLdweights":
                sig = str(inst.ins[0]) + f"|t{inst.is_transpose}"
                si = inst.sync_info
                clean = si is None or (not si.on_wait and not si.on_update)
                if sig == last_sig and clean:
                    drop.append(idx)
                else:
                    last_sig = sig
            elif nm == "InstMatmult":
                pass  # matmuls do not clobber loaded weights
        for idx in reversed(drop):
            del insts[idx]
        removed += len(drop)
    return removed


def _build_module(repeat=1):
    """Build the per-core module. `repeat` > 1 wraps the whole computation
    in a device-side loop (used only by the benchmark to amortize dispatch
    overhead); the graded path uses repeat=1."""
    key = f"nc_{repeat}"
    if key in _CACHE:
        return _CACHE[key]

    nc = bacc.Bacc(
        "TRN2",
        target_bir_lowering=False,
        debug=False,
        enable_asserts=False,
        num_devices=NCORES,
    )

    xt_d = nc.dram_tensor("xt", [D, T], BF16, kind="ExternalInput").ap()
    w1_d = nc.dram_tensor("w1", [D, H], BF16, kind="ExternalInput").ap()
    w3_d = nc.dram_tensor("w3", [D, H], BF16, kind="ExternalInput").ap()
    w2_d = nc.dram_tensor("w2", [H, D], BF16, kind="ExternalInput").ap()
    out_d = nc.dram_tensor("out", [T, D], F32, kind="ExternalOutput").ap()

    with tile.TileContext(nc) as tc:
        with (
            tc.tile_pool(name="wpool", bufs=1) as wpool,
            tc.tile_pool(name="xpool", bufs=2) as xpool,
            tc.tile_pool(name="hpool", bufs=2) as hpool,
            tc.tile_pool(name="spool", bufs=2) as spool,
            tc.tile_pool(name="opool", bufs=4) as opool,
            tc.tile_pool(name="ps", bufs=1, space="PSUM") as ps,
        ):
            def body():
                # Resident weights: distinct tags -> one persistent slot
                # each.
                w1_t = []
                w3_t = []
                for k in range(NDK):
                    t1 = wpool.tile([P, H], BF16, tag=f"w1_{k}")
                    t3 = wpool.tile([P, H], BF16, tag=f"w3_{k}")
                    nc.sync.dma_start(t1[:], w1_d[k * P:(k + 1) * P, :])
                    nc.sync.dma_start(t3[:], w3_d[k * P:(k + 1) * P, :])
                    w1_t.append(t1)
                    w3_t.append(t3)
                w2_t = []
                for k in range(NHM):
                    t2 = wpool.tile([P, D], BF16, tag=f"w2_{k}")
                    nc.sync.dma_start(t2[:], w2_d[k * P:(k + 1) * P, :])
                    w2_t.append(t2)

                for ip in range(NTBLK // 2):
                    ts_e = (2 * ip) * TBLK
                    ts_o = (2 * ip + 1) * TBLK
                    xe_t, xo_t = [], []
                    for k in range(NDK):
                        xe = xpool.tile([P, TBLK], BF16, tag=f"xe_{k}")
                        xo = xpool.tile([P, TBLK], BF16, tag=f"xo_{k}")
                        nc.sync.dma_start(
                            xe[:], xt_d[k * P:(k + 1) * P, ts_e:ts_e + TBLK]
                        )
                        nc.sync.dma_start(
                            xo[:], xt_d[k * P:(k + 1) * P, ts_o:ts_o + TBLK]
                        )
                        xe_t.append(xe)
                        xo_t.append(xo)

                    hts_e, hts_o = [], []
                    for hm in range(NHM):
                        hs = hm * P
                        # Each weight lhsT is loaded once and used by both
                        # token blocks of the pair (LDWEIGHTS deduped
                        # post-compile).
                        pa_e = ps.tile([P, TBLK], F32, tag="b0")
                        pa_o = ps.tile([P, TBLK], F32, tag="b1")
                        for k in range(NDK):
                            w = w1_t[k][:, hs:hs + P]
                            st, sp = (k == 0), (k == NDK - 1)
                            nc.tensor.matmul(
                                pa_e[:], w, xe_t[k][:], start=st, stop=sp
                            )
                            nc.tensor.matmul(
                                pa_o[:], w, xo_t[k][:], start=st, stop=sp
                            )
                        sil_e = spool.tile([P, TBLK], BF16, tag="sil_e")
                        sil_o = spool.tile([P, TBLK], BF16, tag="sil_o")
                        nc.scalar.activation(
                            sil_e[:], pa_e[:],
                            mybir.ActivationFunctionType.Silu,
                        )
                        nc.scalar.activation(
                            sil_o[:], pa_o[:],
                            mybir.ActivationFunctionType.Silu,
                        )
                        pb_e = ps.tile([P, TBLK], F32, tag="b2")
                        pb_o = ps.tile([P, TBLK], F32, tag="b3")
                        for k in range(NDK):
                            w = w3_t[k][:, hs:hs + P]
                            st, sp = (k == 0), (k == NDK - 1)
                            nc.tensor.matmul(
                                pb_e[:], w, xe_t[k][:], start=st, stop=sp
                            )
                            nc.tensor.matmul(
                                pb_o[:], w, xo_t[k][:], start=st, stop=sp
                            )
                        ht_e = hpool.tile([P, TBLK], BF16, tag=f"he_{hm}")
                        ht_o = hpool.tile([P, TBLK], BF16, tag=f"ho_{hm}")
                        nc.vector.tensor_mul(ht_e[:], sil_e[:], pb_e[:])
                        nc.vector.tensor_mul(ht_o[:], sil_o[:], pb_o[:])
                        hts_e.append(ht_e)
                        hts_o.append(ht_o)

                    # m3: one hT stationary slice serves all four D chunks
                    # (4 matmuls per LDWEIGHTS). Groups alternate between
                    # bank quads b0-b3 / b4-b7 so the DVE evacuation of one
                    # group overlaps the next group's chains; the m1/m2
                    # banks (b0-b3) are free by now (ACT/DVE drained them).
                    for bi, (hts, ts) in enumerate(
                        ((hts_e, ts_e), (hts_o, ts_o))
                    ):
                        for tm in range(NTSUB):
                            tsub = ts + tm * P
                            q = 4 * ((bi * NTSUB + tm) % 2)
                            po = []
                            for dn in range(NDN):
                                po_t = ps.tile(
                                    [P, DBLK], F32, tag=f"b{q + dn}"
                                )
                                po.append(po_t)
                            for hk in range(NHM):
                                lhs = hts[hk][:, tm * P:(tm + 1) * P]
                                st, sp = (hk == 0), (hk == NHM - 1)
                                for dn in range(NDN):
                                    nc.tensor.matmul(
                                        po[dn][:], lhs,
                                        w2_t[hk][:, dn * DBLK:(dn + 1) * DBLK],
                                        start=st, stop=sp,
                                    )
                            for dn in range(NDN):
                                ot = opool.tile([P, DBLK], F32, tag="o")
                                nc.vector.tensor_copy(ot[:], po[dn][:])
                                nc.sync.dma_start(
                                    out_d[tsub:tsub + P,
                                          dn * DBLK:(dn + 1) * DBLK],
                                    ot[:],
                                )

            if repeat == 1:
                body()
            else:
                # hint_engines arms the branch prefetcher: the body is far
                # larger than one IRAM block, so the back-edge would
                # otherwise stall ~4us per engine on an I$ miss.
                with tc.For_i(
                    0, repeat, 1,
                    hint_engines=(
                        mybir.EngineType.PE,
                        mybir.EngineType.SP,
                        mybir.EngineType.DVE,
                        mybir.EngineType.Activation,
                    ),
                ):
                    body()

    nc.compile()
    _dedupe_ldweights(nc)
    _CACHE[key] = nc
    return nc


def _stage_inputs(x, w1, w2, w3):
    """Per-expert bf16 staging; x pre-transposed to [D, T]."""
    bf = ml_dtypes.bfloat16
    in_maps = []
    for e in range(E):
        in_maps.append({
            "xt": np.ascontiguousarray(x[e].astype(bf).T),
            "w1": np.ascontiguousarray(w1[e].astype(bf)),
            "w3": np.ascontiguousarray(w3[e].astype(bf)),
            "w2": np.ascontiguousarray(w2[e].astype(bf)),
        })
    return in_maps


def kernel(x, w1, w2, w3):
    assert x.shape == (E, T, D) and w1.shape == (E, D, H)
    assert w2.shape == (E, H, D) and w3.shape == (E, D, H)
    nc = _build_module()
    in_maps = _stage_inputs(x, w1, w2, w3)
    res = run_bass_kernel_spmd(nc, in_maps, core_ids=list(range(NCORES)))
    out = np.stack([res.results[e]["out"] for e in range(E)], axis=0)
    return out.astype(np.float32)


# revision 22
# speedup vs baseline: 1.0326x; 1.0326x over previous
"""GroupedExperts (MoE bmm path) forward on 8 Trainium2 NeuronCores.

Reference (per expert e):
    h   = silu(x[e] @ w1[e]) * (x[e] @ w3[e])
    out = h @ w2[e]
with E=8, T=4096, D=2048, H=1024, fp32 inputs.

Sharding: expert-parallel - core e owns expert e (no cross-core traffic).

Device kernel design (per core):
  Host stages inputs as bf16 with x pre-transposed to xT [D, T] so every
  matmul consumes its natural layout (no on-device transposes):
    m1/m2: aT/bT[hm, tblk] = sum_dk w1/w3[dk, hm].T @ xT[dk, tblk]
           (lhsT = weight tile [128(D) x 128(H)], rhs = xT tile)
    hT    = silu(aT) * bT                     (ACT + DVE, bf16 result)
    m3:    out[tm, dn] = sum_hk hT[hk, tm].T @ w2[hk, dn]
  PSUM accumulates in fp32; out is written fp32 in natural [T, D] layout.
  Weights stay SBUF-resident (~96KB/partition); xT / hT are double-buffered
  per 512-token block so DMA and PE overlap.

  LDWEIGHTS amortization: the PE pays ~50-100 ns per LDWEIGHTS serially
  (not hidden behind matmuls on TRN2), so the loops are structured to
  reuse each stationary operand for two matmuls:
   - m1/m2 process token blocks in pairs: the two blocks' accumulation
     chains interleave per contraction tile (same weight, two rhs
     tiles, two PSUM banks).
   - m3 streams all four 512-wide D chunks per hT stationary slice
     (4 matmuls per LDWEIGHTS), alternating between the two PSUM bank
     quads so DVE evacuation overlaps the next group's chains.
  After Tile compilation, `_dedupe_ldweights` drops the redundant
  (sync-free, identical-AP) InstLdweights - the PE keeps the
  stationary operand resident across consecutive matmuls.
  PSUM budget: all 8 banks, time-multiplexed between the m1/m2 chains
  (b0-b3) and the m3 output groups (alternating b0-b3 / b4-b7).
  SBUF ~204KB/partition (weights 96 + x 64 + h 32 + staging 12).
"""

import numpy as np
import ml_dtypes

import concourse.bass as bass
import concourse.mybir as mybir
import concourse.tile as tile
from concourse import bacc
from concourse.bass_utils import run_bass_kernel_spmd

E, T, D, H = 8, 4096, 2048, 1024
NCORES = 8
P = 128               # partition dim
TBLK = 512            # token block = moving free dim for m1/m2
NTBLK = T // TBLK     # 8
NDK = D // P          # 16 contraction tiles over D
NHM = H // P          # 8 tiles over H
DBLK = 512            # D chunk = moving free dim for m3
NDN = D // DBLK       # 4
NTSUB = TBLK // P     # 4

BF16 = mybir.dt.bfloat16
F32 = mybir.dt.float32

_CACHE: dict = {}


def _dedupe_ldweights(nc):
    """Drop back-to-back InstLdweights with identical weight APs.

    The PE array keeps the stationary operand resident across matmuls;
    a second LDWEIGHTS that (a) loads the same AP as the most recent
    one, (b) carries no semaphore waits/updates, and (c) has no
    intervening weight-clobbering instruction is a no-op and only costs
    PE issue time. Safe only at the final (post-schedule) order.
    """
    removed = 0
    for blk in nc.m.functions[0].blocks:
        insts = blk.instructions
        last_sig = None
        drop = []
        for idx, inst in enumerate(insts):
            nm = inst.__class__.__name__
            if nm == "InstLdweights":
                sig = str(inst.ins[0]) + f"|t{inst.is_transpose}"
                si = inst.sync_info
                clean = si is None or (not si.on_wait and not si.on_update)
                if sig == last_sig and clean:
                    drop.append(idx)
                else:
                    last_sig = sig
            elif nm == "InstMatmult":
                pass  # matmuls do not clobber loaded weights
        for idx in reversed(drop):
            del insts[idx]
        removed += len(drop)
    return removed


def _build_module(repeat=1):
    """Build the per-core module. `repeat` > 1 wraps the whole computation
    in a device-side loop (used only by the benchmark to amortize dispatch
    overhead); the graded path uses repeat=1."""
    key = f"nc_{repeat}"
    if key in _CACHE:
        return _CACHE[key]

    nc = bacc.Bacc(
        "TRN2",
        target_bir_lowering=False,
        debug=False,
        enable_asserts=False,
        num_devices=NCORES,
    )

    xt_d = nc.dram_tensor("xt", [D, T], BF16, kind="ExternalInput").ap()
    w1_d = nc.dram_tensor("w1", [D, H], BF16, kind="ExternalInput").ap()
    w3_d = nc.dram_tensor("w3", [D, H], BF16, kind="ExternalInput").ap()
    w2_d = nc.dram_tensor("w2", [H, D], BF16, kind="ExternalInput").ap()
    out_d = nc.dram_tensor("out", [T, D], F32, kind="ExternalOutput").ap()

    with tile.TileContext(nc) as tc:
        with (
            tc.tile_pool(name="wpool", bufs=1) as wpool,
            tc.tile_pool(name="xpool", bufs=2) as xpool,
            tc.tile_pool(name="hpool", bufs=2) as hpool,
            tc.tile_pool(name="spool", bufs=2) as spool,
            tc.tile_pool(name="opool", bufs=4) as opool,
            tc.tile_pool(name="ps", bufs=1, space="PSUM") as ps,
        ):
            def body():
                # Resident weights: distinct tags -> one persistent slot
                # each.
                # Issue only the w1 loads up front: the first pa chain
                # needs w1 + the first x pair, so w3/w2 loads are deferred
                # behind the first x DMAs (below) - they are not consumed
                # until the first pb chain / m3 phase. This shortens the
                # post-back-edge DMA critical path from ~21MB to ~12.6MB.
                w1_t = []
                w3_t = []
                for k in range(NDK):
                    t1 = wpool.tile([P, H], BF16, tag=f"w1_{k}")
                    t3 = wpool.tile([P, H], BF16, tag=f"w3_{k}")
                    nc.sync.dma_start(t1[:], w1_d[k * P:(k + 1) * P, :])
                    w1_t.append(t1)
                    w3_t.append(t3)
                w2_t = []
                for k in range(NHM):
                    t2 = wpool.tile([P, D], BF16, tag=f"w2_{k}")
                    w2_t.append(t2)

                for ip in range(NTBLK // 2):
                    ts_e = (2 * ip) * TBLK
                    ts_o = (2 * ip + 1) * TBLK
                    xe_t, xo_t = [], []
                    for k in range(NDK):
                        xe = xpool.tile([P, TBLK], BF16, tag=f"xe_{k}")
                        xo = xpool.tile([P, TBLK], BF16, tag=f"xo_{k}")
                        nc.sync.dma_start(
                            xe[:], xt_d[k * P:(k + 1) * P, ts_e:ts_e + TBLK]
                        )
                        nc.sync.dma_start(
                            xo[:], xt_d[k * P:(k + 1) * P, ts_o:ts_o + TBLK]
                        )
                        xe_t.append(xe)
                        xo_t.append(xo)

                    if ip == 0:
                        for k in range(NDK):
                            nc.sync.dma_start(
                                w3_t[k][:], w3_d[k * P:(k + 1) * P, :]
                            )
                        for k in range(NHM):
                            nc.sync.dma_start(
                                w2_t[k][:], w2_d[k * P:(k + 1) * P, :]
                            )

                    hts_e, hts_o = [], []
                    for hm in range(NHM):
                        hs = hm * P
                        # Each weight lhsT is loaded once and used by both
                        # token blocks of the pair (LDWEIGHTS deduped
                        # post-compile).
                        pa_e = ps.tile([P, TBLK], F32, tag="b0")
                        pa_o = ps.tile([P, TBLK], F32, tag="b1")
                        for k in range(NDK):
                            w = w1_t[k][:, hs:hs + P]
                            st, sp = (k == 0), (k == NDK - 1)
                            nc.tensor.matmul(
                                pa_e[:], w, xe_t[k][:], start=st, stop=sp
                            )
                            nc.tensor.matmul(
                                pa_o[:], w, xo_t[k][:], start=st, stop=sp
                            )
                        sil_e = spool.tile([P, TBLK], BF16, tag="sil_e")
                        sil_o = spool.tile([P, TBLK], BF16, tag="sil_o")
                        nc.scalar.activation(
                            sil_e[:], pa_e[:],
                            mybir.ActivationFunctionType.Silu,
                        )
                        nc.scalar.activation(
                            sil_o[:], pa_o[:],
                            mybir.ActivationFunctionType.Silu,
                        )
                        pb_e = ps.tile([P, TBLK], F32, tag="b2")
                        pb_o = ps.tile([P, TBLK], F32, tag="b3")
                        for k in range(NDK):
                            w = w3_t[k][:, hs:hs + P]
                            st, sp = (k == 0), (k == NDK - 1)
                            nc.tensor.matmul(
                                pb_e[:], w, xe_t[k][:], start=st, stop=sp
                            )
                            nc.tensor.matmul(
                                pb_o[:], w, xo_t[k][:], start=st, stop=sp
                            )
                        ht_e = hpool.tile([P, TBLK], BF16, tag=f"he_{hm}")
                        ht_o = hpool.tile([P, TBLK], BF16, tag=f"ho_{hm}")
                        nc.vector.tensor_mul(ht_e[:], sil_e[:], pb_e[:])
                        nc.vector.tensor_mul(ht_o[:], sil_o[:], pb_o[:])
                        hts_e.append(ht_e)
                        hts_o.append(ht_o)

                    # m3: one hT stationary slice serves all four D chunks
                    # (4 matmuls per LDWEIGHTS). Groups alternate between
                    # bank quads b0-b3 / b4-b7 so the DVE evacuation of one
                    # group overlaps the next group's chains; the m1/m2
                    # banks (b0-b3) are free by now (ACT/DVE drained them).
                    for bi, (hts, ts) in enumerate(
                        ((hts_e, ts_e), (hts_o, ts_o))
                    ):
                        for tm in range(NTSUB):
                            tsub = ts + tm * P
                            q = 4 * ((bi * NTSUB + tm) % 2)
                            po = []
                            for dn in range(NDN):
                                po_t = ps.tile(
                                    [P, DBLK], F32, tag=f"b{q + dn}"
                                )
                                po.append(po_t)
                            for hk in range(NHM):
                                lhs = hts[hk][:, tm * P:(tm + 1) * P]
                                st, sp = (hk == 0), (hk == NHM - 1)
                                for dn in range(NDN):
                                    nc.tensor.matmul(
                                        po[dn][:], lhs,
                                        w2_t[hk][:, dn * DBLK:(dn + 1) * DBLK],
                                        start=st, stop=sp,
                                    )
                            for dn in range(NDN):
                                ot = opool.tile([P, DBLK], F32, tag="o")
                                nc.vector.tensor_copy(ot[:], po[dn][:])
                                nc.sync.dma_start(
                                    out_d[tsub:tsub + P,
                                          dn * DBLK:(dn + 1) * DBLK],
                                    ot[:],
                                )

            if repeat == 1:
                body()
            else:
                # hint_engines arms the branch prefetcher: the body is far
                # larger than one IRAM block, so the back-edge would
                # otherwise stall ~4us per engine on an I$ miss.
                with tc.For_i(
                    0, repeat, 1,
                    hint_engines=(
                        mybir.EngineType.PE,
                        mybir.EngineType.SP,
                        mybir.EngineType.DVE,
                        mybir.EngineType.Activation,
                    ),
                    staggered_reset=True,
                ):
                    body()

    nc.compile()
    _dedupe_ldweights(nc)
    _CACHE[key] = nc
    return nc


def _stage_inputs(x, w1, w2, w3):
    """Per-expert bf16 staging; x pre-transposed to [D, T]."""
    bf = ml_dtypes.bfloat16
    in_maps = []
    for e in range(E):
        in_maps.append({
            "xt": np.ascontiguousarray(x[e].astype(bf).T),
            "w1": np.ascontiguousarray(w1[e].astype(bf)),
            "w3": np.ascontiguousarray(w3[e].astype(bf)),
            "w2": np.ascontiguousarray(w2[e].astype(bf)),
        })
    return in_maps


def kernel(x, w1, w2, w3):
    assert x.shape == (E, T, D) and w1.shape == (E, D, H)
    assert w2.shape == (E, H, D) and w3.shape == (E, D, H)
    nc = _build_module()
    in_maps = _stage_inputs(x, w1, w2, w3)
    res = run_bass_kernel_spmd(nc, in_maps, core_ids=list(range(NCORES)))
    out = np.stack([res.results[e]["out"] for e in range(E)], axis=0)
    return out.astype(np.float32)
